# revision 48
# baseline (speedup 1.0000x reference)
"""Trainium2 Bass kernel for GQA attention (B=8, S=1024, H=2048, 32 Q / 8 KV heads, D=64).

Data-parallel over batch: one batch element per NeuronCore, weights
replicated, zero collectives. All wire/weight traffic is bfloat16 (converted
on the host); PSUM accumulation is fp32.

Per-core pipeline:
  1. PE-transpose hidden (bf16) -> hT [H, S] as 16 x [128, S] tiles.
  2. V projection -> va tiles [s, 8*65] bf16 with a ones column at group
     col 64 (PV matmul then accumulates the softmax denominator for free).
  3. K projection (kT layout [kd, s]) + RoPE (bf16 tables, partition-shift
     via 4 small DMAs per tile) -> kT bf16 with both 64-partition slots
     duplicated.
  4. Q projection + RoPE -> written straight into a persistent qT SBUF tile
     (no DRAM spill).
  5. Attention per head: causal scoresT = kT^T q (bf16), diagonal mask added
     via identity-matmul, one exp per key-tile on ScalarE, PV matmul
     accumulates output + denominator. Normalization runs entirely off PE:
     DVE copies PSUM->SBUF, GPSIMD partition-broadcasts the denominator row,
     DVE reciprocal + multiply, DMA into attT (bf16).
  6. O-projection from attT (bf16), PSUM DMA'd straight to DRAM output (f32).

DMA streams are spread across the SP/Act/Pool/DVE queues so no engine's DMA
serialization stalls the PE.
"""

import contextlib

import ml_dtypes
import numpy as np

import concourse.bass as bass
import concourse.tile as tile
from concourse import bacc, mybir
from concourse.bass_utils import run_bass_kernel_spmd

B, S, H = 8, 1024, 2048
NQ, NKV, D = 32, 8, 64
F32 = mybir.dt.float32
BF16 = mybir.dt.bfloat16
NEG = -1.0e30
AF = mybir.ActivationFunctionType
BF = ml_dtypes.bfloat16


def _tables():
    inv = 1.0 / (10000.0 ** (np.arange(0, D, 2, dtype=np.float64) / D))  # [32]
    fr = np.arange(S, dtype=np.float64)[:, None] * inv[None, :]  # [S, 32]
    cos = np.cos(fr).T  # [32, S]
    sin = np.sin(fr).T
    cosT = np.concatenate([cos, cos], 0)  # [64, S]
    sgnT = np.concatenate([-sin, sin], 0)  # [64, S]
    cos128 = np.concatenate([cosT, cosT], 0).astype(BF)  # [128, S]
    sgn128 = np.concatenate([sgnT, sgnT], 0).astype(BF)
    p = np.arange(128)[:, None]
    c = np.arange(128)[None, :]
    mask = np.where(p <= c, 0.0, NEG).astype(BF)  # [128, 128]
    ident = np.eye(128, dtype=BF)
    return cos128, sgn128, mask, ident


def _rope(nc, rp, ps, cos_sl, sgn_sl, out_sl):
    """psum [128,512] f32 (raw qT/kT tile) -> RoPE applied, bf16, into out_sl."""
    raw = rp.tile([128, 512], BF16, name="rope_raw", tag="rope_raw")
    nc.scalar.copy(raw[:], ps[:])
    sh = rp.tile([128, 512], BF16, name="rope_sh", tag="rope_sh")
    for a in range(4):  # partition quarter a reads quarter a^1  (p -> p xor 32)
        sc = (a ^ 1) * 32
        eng = nc.sync if a % 2 == 0 else nc.gpsimd
        eng.dma_start(out=sh[a * 32 : (a + 1) * 32, :], in_=raw[sc : sc + 32, :])
    tmp = rp.tile([128, 512], BF16, name="rope_tmp", tag="rope_tmp")
    nc.vector.tensor_mul(tmp[:], raw[:], cos_sl)
    rot = rp.tile([128, 512], BF16, name="rope_rot", tag="rope_rot")
    nc.gpsimd.tensor_mul(rot[:], sh[:], sgn_sl)
    nc.vector.tensor_add(out_sl, tmp[:], rot[:])


def _body(nc, tc, ctx, hid, wq, wk, wv, wo, cosd, sgnd, maskd, identd, onesd, outd):
    # ---- constants (live whole body) ----
    cpool = ctx.enter_context(tc.tile_pool(name="const", bufs=1))
    ident_b = cpool.tile([128, 128], BF16, name="ident_b", tag="ident_b")
    nc.sync.dma_start(ident_b[:], identd[:])
    mask_b = cpool.tile([128, 128], BF16, name="mask_b", tag="mask_b")
    nc.gpsimd.dma_start(out=mask_b[:], in_=maskd[:])

    # persistent right-side tiles: va, kT, qT (live through attention)
    attn_ctx = contextlib.ExitStack()
    vapool = attn_ctx.enter_context(tc.tile_pool(name="vap", bufs=1, side="right"))
    va = [
        vapool.tile([128, 8 * 65], BF16, name=f"va{s}", tag=f"va{s}")
        for s in range(8)
    ]
    kpool = attn_ctx.enter_context(tc.tile_pool(name="kTp", bufs=1, side="right"))
    kT = kpool.tile([128, 8 * S], BF16, name="kT", tag="kT")
    qpool = attn_ctx.enter_context(tc.tile_pool(name="qTp", bufs=1, side="right"))
    qT = qpool.tile([128, 16 * S], BF16, name="qT", tag="qT")

    with contextlib.ExitStack() as proj_ctx:
        tabp = proj_ctx.enter_context(tc.tile_pool(name="ropetab", bufs=1))
        cos128 = tabp.tile([128, S], BF16, name="cos", tag="cos")
        nc.scalar.dma_start(cos128[:], cosd[:])
        sgn128 = tabp.tile([128, S], BF16, name="sgn", tag="sgn")
        nc.scalar.dma_start(sgn128[:], sgnd[:])
        # shared weight-chunk pool: wv/wk/wq stream [128, 8*512] bf16 chunks
        wbufp = proj_ctx.enter_context(tc.tile_pool(name="wbuf", bufs=4))
        hTpool = proj_ctx.enter_context(tc.tile_pool(name="hTp", bufs=1))
        hT = [hTpool.tile([128, S], BF16, name=f"hT{c}", tag=f"hT{c}") for c in range(16)]

        # ================= Phase 1: transpose hidden =================
        # 8 row-tiles of hidden, each loaded as two half-column DMAs spread
        # over 4 queues so transfers overlap.
        engs = [nc.sync, nc.scalar, nc.gpsimd]
        with tc.tile_pool(name="hidnat", bufs=8) as hp, tc.tile_pool(
            name="tpsum", bufs=6, space="PSUM"
        ) as tp:
            hid_nat = [
                hp.tile([128, H], BF16, name="hidnat", tag="hidnat")
                for _ in range(8)
            ]
            # issue order: all low-column halves first (the first transposes
            # need tiles 0-3 at once), spread across the three DMA queues
            k = 0
            for hcol in range(2):
                for t in range(8):
                    engs[k % 3].dma_start(
                        hid_nat[t][:, hcol * 1024 : (hcol + 1) * 1024],
                        hid[t * 128 : (t + 1) * 128, hcol * 1024 : (hcol + 1) * 1024],
                    )
                    k += 1
            # wv prefetch (phase 2) behind the hid loads
            wv_t = []
            for c in range(2):
                wvm = wbufp.tile([128, 8 * 512], BF16, name="wvm", tag="wchunk")
                eng = nc.sync if c == 0 else nc.scalar
                eng.dma_start(
                    wvm.rearrange("p (t f) -> p t f", t=8),
                    wv.rearrange("(t p) f -> p t f", p=128)[:, c * 8 : c * 8 + 8],
                )
                wv_t += [wvm[:, h * 512 : (h + 1) * 512] for h in range(8)]
            for half in range(2):
                for c in range(16):
                    ps = tp.tile([128, 512], BF16, name="tp", tag="tp")
                    for tt in range(4):
                        t = half * 4 + tt
                        nc.tensor.transpose(
                            ps[:, tt * 128 : (tt + 1) * 128],
                            hid_nat[t][:, c * 128 : (c + 1) * 128],
                            ident_b[:],
                        )
                    dst = hT[c][:, half * 512 : (half + 1) * 512]
                    if (half * 16 + c) % 2 == 0:
                        nc.scalar.copy(dst, ps[:])
                    else:
                        nc.vector.tensor_copy(dst, ps[:])

        # ================= Phase 2: V projection (+ ones col) =========
        with tc.tile_pool(name="vpsum", bufs=4, space="PSUM") as vps:
            # wk prefetch (phase 3)
            wk_t = []
            for c in range(2):
                wkm = wbufp.tile([128, 8 * 512], BF16, name="wkm", tag="wchunk")
                eng = nc.sync if c == 0 else nc.scalar
                eng.dma_start(
                    wkm.rearrange("p (t f) -> p t f", t=8),
                    wk.rearrange("(t p) f -> p t f", p=128)[:, c * 8 : c * 8 + 8],
                )
                wk_t += [wkm[:, h * 512 : (h + 1) * 512] for h in range(8)]
            for st in range(8):
                ps = vps.tile([128, 512], F32, name="vp", tag="vp")
                for h in range(16):
                    nc.tensor.matmul(
                        ps[:],
                        hT[h][:, st * 128 : (st + 1) * 128],
                        wv_t[h],
                        start=(h == 0),
                        stop=(h == 15),
                    )
                va3 = va[st].rearrange("p (g c) -> p g c", c=65)
                nc.scalar.copy(
                    va3[:, :, 0:64], ps[:].rearrange("p (g c) -> p g c", c=64)
                )
                nc.gpsimd.dma_start(
                    out=va3[:, :, 64:65],
                    in_=onesd[st * 128 : (st + 1) * 128, :].rearrange(
                        "p (g c) -> p g c", c=1
                    ),
                )

        # ============ Phase 3: K projection + RoPE + slot duplication ==
        with tc.tile_pool(
            name="kpsum", bufs=4, space="PSUM"
        ) as kps, tc.tile_pool(name="krope", bufs=4) as krp, tc.tile_pool(
            name="kfinp", bufs=4
        ) as kfp:
            # wq prefetch (first 2 chunks of phase 4)
            wq_t0 = []
            for c in range(2):
                wqm = wbufp.tile([128, 8 * 512], BF16, name="wqm", tag="wchunk")
                eng = nc.sync if c == 0 else nc.scalar
                eng.dma_start(
                    wqm.rearrange("p (t f) -> p t f", t=8),
                    wq.rearrange("(t p) f -> p t f", p=128)[
                        :, c * 8 : c * 8 + 8, 0:512
                    ],
                )
                wq_t0 += [wqm[:, h * 512 : (h + 1) * 512] for h in range(8)]
            for ft in range(4):
                for ih in range(2):
                    ps = kps.tile([128, 512], F32, name="kp", tag="kp")
                    for h in range(16):
                        nc.tensor.matmul(
                            ps[:],
                            wk_t[h][:, ft * 128 : (ft + 1) * 128],
                            hT[h][:, ih * 512 : (ih + 1) * 512],
                            start=(h == 0),
                            stop=(h == 15),
                        )
                    sl = slice(ih * 512, (ih + 1) * 512)
                    kfin = kfp.tile([128, 512], BF16, name="kfin", tag="kfin")
                    _rope(nc, krp, ps, cos128[:, sl], sgn128[:, sl], kfin[:])
                    b0, b1 = 2 * ft, 2 * ft + 1
                    o0 = b0 * S + ih * 512
                    o1 = b1 * S + ih * 512
                    nc.scalar.dma_start(kT[0:64, o0 : o0 + 512], kfin[0:64, :])
                    nc.gpsimd.dma_start(out=kT[64:128, o0 : o0 + 512], in_=kfin[0:64, :])
                    nc.scalar.dma_start(kT[64:128, o1 : o1 + 512], kfin[64:128, :])
                    nc.gpsimd.dma_start(out=kT[0:64, o1 : o1 + 512], in_=kfin[64:128, :])

        # ========= Phase 4: Q projection + RoPE -> qT in SBUF ==========
        with tc.tile_pool(
            name="qpsum", bufs=4, space="PSUM"
        ) as qps, tc.tile_pool(name="qrope", bufs=4) as qrp:
            for wh in range(4):
                if wh == 0:
                    wq_t = wq_t0
                else:
                    wq_t = []
                    for c in range(2):
                        wqm = wbufp.tile([128, 8 * 512], BF16, name="wqm", tag="wchunk")
                        eng = nc.sync if c == 0 else nc.scalar
                        eng.dma_start(
                            wqm.rearrange("p (t f) -> p t f", t=8),
                            wq.rearrange("(t p) f -> p t f", p=128)[
                                :, c * 8 : c * 8 + 8, wh * 512 : (wh + 1) * 512
                            ],
                        )
                        wq_t += [wqm[:, h * 512 : (h + 1) * 512] for h in range(8)]
                for ftl in range(4):
                    ft = wh * 4 + ftl
                    for ih in range(2):
                        ps = qps.tile([128, 512], F32, name="qp", tag="qp")
                        for h in range(16):
                            nc.tensor.matmul(
                                ps[:],
                                wq_t[h][:, ftl * 128 : (ftl + 1) * 128],
                                hT[h][:, ih * 512 : (ih + 1) * 512],
                                start=(h == 0),
                                stop=(h == 15),
                            )
                        sl = slice(ih * 512, (ih + 1) * 512)
                        off = ft * S + ih * 512
                        _rope(nc, qrp, ps, cos128[:, sl], sgn128[:, sl], qT[:, off : off + 512])

    # hT/tables/weight bufs freed here; va/kT/qT still open
    # ================= Phase 5: attention =================
    apool = ctx.enter_context(tc.tile_pool(name="attTp", bufs=1))
    attT = apool.tile([128, 16 * S], BF16, name="attT", tag="attT")
    # Wo chunk stream: all 8 chunks prefetch during attention, one per early
    # head on gpsimd so per-head normalization DMAs are never queued behind
    # a long weight transfer.
    wop = ctx.enter_context(tc.tile_pool(name="wo", bufs=8))
    wo_t = [
        wop.tile([128, 8 * 512], BF16, name="wom", tag="wom") for _ in range(8)
    ]

    def _wo_load(i):
        ho, c = divmod(i, 2)
        nc.gpsimd.dma_start(
            out=wo_t[i].rearrange("p (t f) -> p t f", t=8),
            in_=wo.rearrange("(t p) f -> p t f", p=128)[
                :, c * 8 : c * 8 + 8, ho * 512 : (ho + 1) * 512
            ],
        )

    with tc.tile_pool(name="scpsum", bufs=2, space="PSUM") as scp, tc.tile_pool(
        name="pvpsum", bufs=2, space="PSUM"
    ) as pvp, tc.tile_pool(name="expT", bufs=5) as exp_p, tc.tile_pool(
        name="pvsb", bufs=3
    ) as pvsbp, tc.tile_pool(name="dbp", bufs=2) as dbp, tc.tile_pool(
        name="pvnp", bufs=3
    ) as pvnp:
        def _norm_tail(pend):
            # one-head-delayed normalization, entirely on Pool: broadcast the
            # raw denominator row, elementwise divide, DMA into attT. DVE does
            # nothing here so its counting semaphore (which gates PV-psum slot
            # reuse) only tracks the prompt pvs copies.
            p_dstg, p_pvs, p_slot, p_bq = pend
            dbr = dbp.tile([64, 1024], F32, name="dbr", tag="dbr")
            nc.gpsimd.partition_broadcast(dbr[:], p_dstg[:], channels=64)
            rrb = dbp.tile([64, 1024], F32, name="rrb", tag="rrb")
            nc.vector.reciprocal_approx_fast(rrb[:], dbr[:])
            pvn = pvnp.tile([64, 1024], BF16, name="pvn", tag="pvn")
            nc.gpsimd.tensor_mul(pvn[:], p_pvs[0:64, :], rrb[:])
            nc.gpsimd.dma_start(
                out=attT[p_slot : p_slot + 64, p_bq * S : p_bq * S + 1024],
                in_=pvn[:],
            )

        # Per-head emission is software-pipelined on the PE: the first two
        # score tiles of head h are emitted BEFORE head h-1's final PV pair,
        # so the Act engine's exp stream never drains at a head boundary.
        pending = None  # (dstg, pvs, slot, bq) awaiting _norm_tail
        pv_tail = None  # closure emitting head h-1's pair67 PVs + pvs/dstg

        def _sc_single(sc, jt, kap, qap):
            lo = jt * 128
            nc.tensor.matmul(
                sc[:, lo:512], kap, qap[:, lo:512],
                start=True, stop=False, skip_group_check=True,
            )
            nc.tensor.matmul(
                sc[:, 512:1024], kap, qap[:, 512:1024],
                start=True, stop=True, skip_group_check=True,
            )
            nc.tensor.matmul(
                sc[:, lo : lo + 128], ident_b[:], mask_b[:],
                start=False, stop=True, skip_group_check=True,
            )
            ex = exp_p.tile([128, 1024], BF16, name="ex", tag="ex")
            nc.scalar.activation(ex[:, lo:1024], sc[:, lo:1024], AF.Exp, scale=0.125)
            return ex

        def _sc_pair(sc, ja, jb, kap_a, kap_b, qap):
            wa = 1024 - 128 * ja
            wb = 1024 - 128 * jb
            nc.tensor.matmul(
                sc[:, 0:wa], kap_a, qap[:, 128 * ja : 1024],
                start=True, stop=False, skip_group_check=True,
            )
            # ja==4 pair: jb's range is in the second PSUM bank -> start=True.
            # ja==6 pair: jb shares ja's bank; a second start=True would
            # re-mark the whole 2KB zero-region pending and turn the
            # ja-diagonal mask accumulate into an overwrite. start=False:
            # the bytes are already pending from ja's start, so this still
            # writes (not accumulates).
            nc.tensor.matmul(
                sc[:, wa : wa + wb], kap_b, qap[:, 128 * jb : 1024],
                start=(ja == 4), stop=False, skip_group_check=True,
            )
            nc.tensor.matmul(
                sc[:, 0:128], ident_b[:], mask_b[:],
                start=False, stop=True, skip_group_check=True,
            )
            nc.tensor.matmul(
                sc[:, wa : wa + 128], ident_b[:], mask_b[:],
                start=False, stop=True, skip_group_check=True,
            )
            ex = exp_p.tile([128, 1024], BF16, name="ex", tag="ex")
            nc.scalar.activation(
                ex[:, 0 : wa + wb], sc[:, 0 : wa + wb], AF.Exp, scale=0.125
            )
            return ex

        for bq in range(16):
            for hs in range(2):
                h = 2 * bq + hs
                g = h // 4
                slot = 64 * hs
                if h < 8:
                    _wo_load(h)
                qap = qT[slot : slot + 64, bq * S : bq * S + 1024]

                def _vab(jt, g=g):
                    return va[jt].rearrange("p (g c) -> p g c", c=65)[:, g, :]

                def _kap(jt, slot=slot, g=g):
                    lo = jt * 128
                    return kT[slot : slot + 64, g * S + lo : g * S + lo + 128]

                # front: score tiles jt0/jt1 + their exps
                sc0 = scp.tile([128, 1024], F32, name="sc", tag="sc")
                ex0 = _sc_single(sc0, 0, _kap(0), qap)
                sc1 = scp.tile([128, 1024], F32, name="sc", tag="sc")
                ex1 = _sc_single(sc1, 1, _kap(1), qap)
                # head h-2's normalization (its dstg DMA landed long ago),
                # then head h-1's deferred PV pair which re-arms `pending`.
                if pending is not None:
                    _norm_tail(pending)
                    pending = None
                if pv_tail is not None:
                    pv_tail()
                # middle
                pv = pvp.tile([65, 1024], F32, name="pv", tag="pv")
                nc.tensor.matmul(
                    pv[:, 0:512], _vab(0), ex0[:, 0:512],
                    start=True, stop=False, skip_group_check=True,
                )
                nc.tensor.matmul(
                    pv[:, 512:1024], _vab(0), ex0[:, 512:1024],
                    start=True, stop=False, skip_group_check=True,
                )
                sc2 = scp.tile([128, 1024], F32, name="sc", tag="sc")
                ex2 = _sc_single(sc2, 2, _kap(2), qap)
                nc.tensor.matmul(
                    pv[:, 128:512], _vab(1), ex1[:, 128:512],
                    start=False, stop=False, skip_group_check=True,
                )
                nc.tensor.matmul(
                    pv[:, 512:1024], _vab(1), ex1[:, 512:1024],
                    start=False, stop=False, skip_group_check=True,
                )
                # tri-merge: jt3 (640 wide) + jt6 (256) + jt7 (128) share one
                # sc tile, columns [0:640]+[640:896]+[896:1024] = one exp.
                sc3 = scp.tile([128, 1024], F32, name="sc", tag="sc")
                nc.tensor.matmul(
                    sc3[:, 0:512], _kap(3), qap[:, 384:896],
                    start=True, stop=False, skip_group_check=True,
                )
                nc.tensor.matmul(
                    sc3[:, 512:640], _kap(3), qap[:, 896:1024],
                    start=True, stop=False, skip_group_check=True,
                )
                nc.tensor.matmul(
                    sc3[:, 640:896], _kap(6), qap[:, 768:1024],
                    start=False, stop=False, skip_group_check=True,
                )
                nc.tensor.matmul(
                    sc3[:, 896:1024], _kap(7), qap[:, 896:1024],
                    start=False, stop=False, skip_group_check=True,
                )
                nc.tensor.matmul(
                    sc3[:, 0:128], ident_b[:], mask_b[:],
                    start=False, stop=False, skip_group_check=True,
                )
                nc.tensor.matmul(
                    sc3[:, 640:768], ident_b[:], mask_b[:],
                    start=False, stop=False, skip_group_check=True,
                )
                nc.tensor.matmul(
                    sc3[:, 896:1024], ident_b[:], mask_b[:],
                    start=False, stop=True, skip_group_check=True,
                )
                ex3 = exp_p.tile([128, 1024], BF16, name="ex", tag="ex")
                nc.scalar.activation(ex3[:], sc3[:], AF.Exp, scale=0.125)
                nc.tensor.matmul(
                    pv[:, 256:512], _vab(2), ex2[:, 256:512],
                    start=False, stop=False, skip_group_check=True,
                )
                nc.tensor.matmul(
                    pv[:, 512:1024], _vab(2), ex2[:, 512:1024],
                    start=False, stop=False, skip_group_check=True,
                )
                sc45 = scp.tile([128, 1024], F32, name="sc", tag="sc")
                ex45 = _sc_pair(sc45, 4, 5, _kap(4), _kap(5), qap)
                nc.tensor.matmul(
                    pv[:, 384:512], _vab(3), ex3[:, 0:128],
                    start=False, stop=True, skip_group_check=True,
                )
                nc.tensor.matmul(
                    pv[:, 512:896], _vab(3), ex3[:, 128:512],
                    start=False, stop=False, skip_group_check=True,
                )
                nc.tensor.matmul(
                    pv[:, 896:1024], _vab(3), ex3[:, 512:640],
                    start=False, stop=False, skip_group_check=True,
                )
                nc.tensor.matmul(
                    pv[:, 512:1024], _vab(4), ex45[:, 0:512],
                    start=False, stop=False, skip_group_check=True,
                )
                nc.tensor.matmul(
                    pv[:, 640:1024], _vab(5), ex45[:, 512:896],
                    start=False, stop=False, skip_group_check=True,
                )

                def _tail(pv=pv, ex3=ex3, vab6=_vab(6), vab7=_vab(7),
                          slot=slot, bq=bq):
                    nonlocal pending
                    nc.tensor.matmul(
                        pv[:, 768:1024], vab6, ex3[:, 640:896],
                        start=False, stop=False, skip_group_check=True,
                    )
                    nc.tensor.matmul(
                        pv[:, 896:1024], vab7, ex3[:, 896:1024],
                        start=False, stop=True, skip_group_check=True,
                    )
                    pvs = pvsbp.tile([65, 1024], F32, name="pvs", tag="pvs")
                    nc.vector.tensor_copy(pvs[:], pv[:])
                    dstg = dbp.tile([1, 1024], F32, name="dstg", tag="dstg")
                    nc.sync.dma_start(dstg[:], pvs[64:65, :])
                    pending = (dstg, pvs, slot, bq)

                pv_tail = _tail
        _norm_tail(pending)
        pending = None
        pv_tail()
        _norm_tail(pending)

    attn_ctx.close()  # free va, kT, qT

    # ================= Phase 6: O projection ================
    with tc.tile_pool(name="opsum", bufs=4, space="PSUM") as ops, tc.tile_pool(
        name="osb", bufs=4
    ) as osbp:
        for ho in range(4):
            woc = []
            for c in range(2):
                wom = wo_t[ho * 2 + c]
                woc += [wom[:, ft * 512 : (ft + 1) * 512] for ft in range(8)]
            for st in range(8):
                ps = ops.tile([128, 512], F32, name="op", tag="op")
                for ft in range(16):
                    nc.tensor.matmul(
                        ps[:],
                        attT[:, ft * S + st * 128 : ft * S + st * 128 + 128],
                        woc[ft],
                        start=(ft == 0),
                        stop=(ft == 15),
                    )
                ob = osbp.tile([128, 512], F32, name="ob", tag="ob")
                nc.scalar.copy(ob[:], ps[:])
                oeng = nc.gpsimd if st % 2 == 0 else nc.sync
                oeng.dma_start(
                    out=outd[st * 128 : (st + 1) * 128, ho * 512 : (ho + 1) * 512],
                    in_=ob[:],
                )


def _build(niter=1):
    nc = bacc.Bacc(None, target_bir_lowering=False)
    hid = nc.declare_dram_parameter("hidden_states", [S, H], BF16, isOutput=False)
    wq = nc.declare_dram_parameter("Wq", [H, NQ * D], BF16, isOutput=False)
    wk = nc.declare_dram_parameter("Wk", [H, NKV * D], BF16, isOutput=False)
    wv = nc.declare_dram_parameter("Wv", [H, NKV * D], BF16, isOutput=False)
    wo = nc.declare_dram_parameter("Wo", [NQ * D, H], BF16, isOutput=False)
    cosd = nc.declare_dram_parameter("rope_cos", [128, S], BF16, isOutput=False)
    sgnd = nc.declare_dram_parameter("rope_sgnsin", [128, S], BF16, isOutput=False)
    maskd = nc.declare_dram_parameter("causal_mask", [128, 128], BF16, isOutput=False)
    identd = nc.declare_dram_parameter("ident", [128, 128], BF16, isOutput=False)
    onesd = nc.declare_dram_parameter("ones_col", [S, 8], BF16, isOutput=False)
    outd = nc.declare_dram_parameter("out", [S, H], F32, isOutput=True)

    with tile.TileContext(nc) as tc:
        for _ in range(niter):
            with contextlib.ExitStack() as ctx:
                _body(nc, tc, ctx, hid, wq, wk, wv, wo, cosd, sgnd, maskd, identd, onesd, outd)
    nc.compile()
    return nc


_CACHE = {}


def _get_nc(niter=1):
    if niter not in _CACHE:
        _CACHE[niter] = _build(niter)
    return _CACHE[niter]


def _in_maps(inputs):
    cos128, sgn128, mask, ident = _tables()
    hidden = np.asarray(inputs["hidden_states"], dtype=np.float32).astype(BF)
    base = {
        "Wq": np.ascontiguousarray(np.asarray(inputs["Wq"], np.float32).astype(BF)),
        "Wk": np.ascontiguousarray(np.asarray(inputs["Wk"], np.float32).astype(BF)),
        "Wv": np.ascontiguousarray(np.asarray(inputs["Wv"], np.float32).astype(BF)),
        "Wo": np.ascontiguousarray(np.asarray(inputs["Wo"], np.float32).astype(BF)),
        "rope_cos": cos128,
        "rope_sgnsin": sgn128,
        "causal_mask": mask,
        "ident": ident,
        "ones_col": np.ones((S, 8), BF),
    }
    return [dict(base, hidden_states=np.ascontiguousarray(hidden[i])) for i in range(B)]


def kernel(**inputs):
    nc = _get_nc(1)
    res = run_bass_kernel_spmd(nc, _in_maps(inputs), core_ids=list(range(8)))
    return np.stack(
        [np.asarray(res.results[i]["out"], dtype=np.float32) for i in range(B)]
    )
